# revision 39
# baseline (speedup 1.0000x reference)
"""DePatchEfficient Trainium2 kernel.

Reconstructs a (B, U, V, S, T, C) volume from overlapping 4D patches by
scatter-add + overlap-count division (overlap-add).

Decomposition: polyphase overlap-add. Split patch offsets ju = 2a + ru,
jv = 2b + rv, js = 4e + ws, jt = 4f + wt. Then every output element
  out[u=2mu+ru, v=2mv+rv, s=4qs+ws, t=4qt+wt, c]
is the sum over 16 terms (a, b, e, f) of shifted input slabs with
mu = iu + a, mv = iv + b, qs = is + e, qt = it + f.

Design notes (measured on HW: DMA engine time is paced by SBUF-landed
bytes at ~19.5 GB/s/engine x 16 engines; DVE adds at ~2 elem/cyc fp16):
- Inputs ship as int8: q = round(x / s) with one scale s per core. The
  Pool-engine (SWDGE) DMA casts int8 -> fp16 in flight. All on-core
  adds are integer-valued fp16 sums bounded by 16*127 = 2032 < 2048 --
  EXACT in fp16. The only error is input quantization (~s/2 = 0.02 abs,
  ~5.7e-3 rel vs the output max), well under the 2e-2 gate.
- Both the scale s and the 1/overlap-count division fold into the
  host-side assembly (a single fused multiply on the fp32 result).
- Two e=0 terms stage as raw int8 in SBUF: their DMA moves half the
  bytes while their adds drop to DVE 1x mode -- balances the DMA stream
  (the pacer) against DVE slack.
- Partition dim is (qt', ws, wt) = 128 with qt' = 7 - qt: the f-shift
  becomes a partition-range restriction starting at 0 (SBUF APs cannot
  start at partition 16), so only the e-shift still forces the free AP
  over 3 dims (e=1 terms loop over mu).
- The acc is zeroed by one DVE memset in the dead window while the
  first DMA is in flight; every term is then a plain add. The first
  term's DMA is split by iu so adds start early; the final term's add
  is split by mu so the output DMA pipelines behind the last adds.

Sharding: 8 cores = (batch b2) x (s-half) x (t-half); high halves are
axis-flipped host-side so all cores run an identical program. Halo
elements are clipped host-side: each input element ships to one core.

On-core layout (SBUF): partitions = (qt'8, ws4, wt4) = 128; free dim =
(mu8, mv8, qs8, ru2, rv2, c3) = 6144 fp16. ru/rv/c form a never-sliced
contiguous 12-elem inner block, keeping fp16 adds in DVE 2x_1p mode.
"""

import os
import sys

import numpy as np

for _p in ("/opt/trn_rl_repo",):
    if os.path.isdir(_p) and _p not in sys.path:
        sys.path.insert(0, _p)

B, U, V, S, T, C = 2, 16, 16, 64, 64, 3
NS, NT, NU, NV = 15, 15, 7, 7
P128 = 128        # partitions: (qt'8, ws4, wt4)
FREE = 6144       # free dim: (mu8, mv8, qs8, ru2, rv2, c3)

# The acc is zeroed by one full memset in the dead DVE window while the
# first DMA is in flight, so every term is a plain add in any order.
# Smallest terms (e=1, f=1) go first so the first add starts earliest;
# (0,0,1,1) is last: its add is mu-split so the output DMA pipelines
# behind the final adds.
TERMS = [
    (1, 1, 0, 0), (1, 1, 0, 1), (1, 1, 1, 0), (1, 1, 1, 1),
    (1, 0, 0, 0), (1, 0, 0, 1), (1, 0, 1, 0), (1, 0, 1, 1),
    (0, 0, 0, 0), (0, 0, 0, 1), (0, 0, 1, 0), (0, 1, 0, 0),
    (0, 1, 0, 1), (0, 1, 1, 0), (0, 1, 1, 1), (0, 0, 1, 1),
]

# Terms staged as raw int8 in SBUF: their DMA moves half the bytes
# (the stream is dest-byte-paced) while their DVE add drops to 1x mode
# — DVE has slack, the DMA stream is the critical path. Balanced at ~2
# single-op (e=0) terms.
INT8_TERMS = {(0, 0, 0, 1), (0, 1, 0, 0)}

# Per-axis overlap counts in core-local coordinates (after flips every
# core sees the volume edge at index 0; u/v are unsharded so both ends
# are edges).
_CU = np.array([1, 2, 2, 2, 2, 2, 2, 1], np.float32)   # mu / mv
_CS = np.array([1, 2, 2, 2, 2, 2, 2, 2], np.float32)   # qs / qt


def _term_name(e, f, a, b):
    return f"t{e}{f}{a}{b}"


def _inv_count_map():
    """1/count over the core-local assembled slab (U, V, 32, 32, 1)."""
    cu = _CU[np.arange(U) // 2]            # (16,) by u
    cs = _CS[np.arange(32) // 4]           # (32,) by s_local
    cnt = (cu[:, None, None, None] * cu[None, :, None, None]
           * cs[None, None, :, None] * cs[None, None, None, :])
    return (1.0 / cnt[..., None]).astype(np.float32)


_INV_COUNT = _inv_count_map()


def _shard(x):
    """Full input (B, 11025, 4, 4, 8, 8, 3) -> (per-core in_maps, scales).

    Per core: quantize to int8 with scale s = max|x_core|/127; terms are
    staged so the SWDGE cast-DMA lands them as fp16 integer values."""
    x9 = np.ascontiguousarray(x).reshape(B, NS, NT, NU, NV, 4, 4, 8, 8, C)
    in_maps, scales = [], []
    for core in range(8):
        b, sh, th = core // 4, (core // 2) % 2, core % 2
        xc = x9[b, 7 * sh:7 * sh + 8, 7 * th:7 * th + 8]
        # Flip high-half cores so every core sees an "s/t low half" problem.
        if sh:
            xc = xc[::-1, :, :, :, :, :, ::-1]
        if th:
            xc = xc[:, ::-1, :, :, :, :, :, ::-1]
        s = float(np.abs(xc).max()) / 127.0
        # (is, it, iu, iv, a ru, b rv, e ws, f wt, c)
        xq = np.clip(np.rint(xc * (1.0 / s)), -127, 127).astype(np.int8)
        xq = xq.reshape(8, 8, NU, NV, 2, 2, 2, 2, 2, 4, 2, 4, C)
        scales.append(s)
        m = {}
        for (e, f, a, bb) in TERMS:
            isN, itN = 8 - e, 8 - f
            sl = xq[:isN, :itN, :, :, a, :, bb, :, e, :, f, :, :]
            # it axis reversed: acc partitions hold qt' = 7 - qt, so every
            # term's partition window starts at 0.
            sl = sl[:, ::-1]
            # (is,it',iu,iv,ru,rv,ws,wt,c) -> (it',ws,wt,iu,iv,is,ru,rv,c)
            sl = sl.transpose(1, 6, 7, 2, 3, 0, 4, 5, 8)
            m[_term_name(e, f, a, bb)] = np.ascontiguousarray(
                sl.reshape(itN * 16, NU * NV * isN * 12)
            )
        # First term's first 3 iu-blocks also ship as fp16 for the HWDGE
        # (sync) queue, which cannot cast: the first add starts before
        # the SWDGE pipeline spins up.
        e0, f0, a0, b0 = TERMS[0]
        cs0 = NV * (8 - e0) * 12 * 2
        m["t_first"] = m[_term_name(e0, f0, a0, b0)][:, :cs0].astype(np.float16)
        # zeros for acc partitions 112:128 (written by DMA: SBUF memset
        # APs cannot start at partition 112, DMAs can)
        m["zrows"] = np.zeros((16, FREE), np.float16)
        in_maps.append(m)
    return in_maps, scales


def _assemble(core_outs, scales):
    """Per-core (128, 6144) fp16 int-sums -> full (B, U, V, S, T, C) fp32."""
    full = np.empty((B, U, V, S, T, C), np.float32)
    for core in range(8):
        b, sh, th = core // 4, (core // 2) % 2, core % 2
        o = core_outs[core].reshape(8, 4, 4, 8, 8, 8, 2, 2, C)[::-1]
        # (qt, ws, wt, mu, mv, qs, ru, rv, c) -> (mu ru, mv rv, qs ws, qt wt, c)
        o = o.transpose(3, 6, 4, 7, 5, 1, 0, 2, 8).reshape(U, V, 32, 32, C)
        o = o.astype(np.float32) * (scales[core] * _INV_COUNT)
        if sh:
            o = o[:, :, ::-1]
        if th:
            o = o[:, :, :, ::-1]
        full[b, :, :, 32 * sh:32 * sh + 32, 32 * th:32 * th + 32, :] = o
    return full


def build_nc(reps=1):
    """Build the per-core Bass program (identical for all 8 cores).

    reps>1 wraps the whole pass in a Tile For_i loop executing it `reps`
    times back-to-back — benchmark-only; the graded kernel() path uses
    reps=1 with no loop.
    """
    import concourse.bacc as bacc
    import concourse.mybir as mybir
    from concourse.tile import TileContext

    nc = bacc.Bacc("TRN2", target_bir_lowering=False, debug=False)
    terms = {
        (e, f, a, b): nc.dram_tensor(
            _term_name(e, f, a, b),
            [(8 - f) * 16, NU * NV * (8 - e) * 12],
            mybir.dt.int8,
            kind="ExternalInput",
        )
        for (e, f, a, b) in TERMS
    }
    e0, f0 = TERMS[0][0], TERMS[0][1]
    tfirst = nc.dram_tensor(
        "t_first", [(8 - f0) * 16, NV * (8 - e0) * 12 * 2], mybir.dt.float16,
        kind="ExternalInput",
    )
    zrows = nc.dram_tensor("zrows", [16, FREE], mybir.dt.float16,
                           kind="ExternalInput")
    out = nc.dram_tensor("out", [P128, FREE], mybir.dt.float16, kind="ExternalOutput")

    from contextlib import ExitStack

    with (
        TileContext(nc) as tc,
        tc.tile_pool(name="accp", bufs=1) as accp,
        tc.tile_pool(name="stgp", bufs=6) as stgp,
        tc.tile_pool(name="stg8p", bufs=2) as stg8p,
        ExitStack() as stack,
    ):
        if reps > 1:
            stack.enter_context(tc.For_i(0, reps, 1))
        if True:
            acc = accp.tile([P128, FREE], mybir.dt.float16)
            accv = acc[:, :].rearrange(
                "p (mu mv qs ru rv c) -> p mu mv qs ru rv c",
                mu=8, mv=8, qs=8, ru=2, rv=2, c=3,
            )
            # Zero the acc while the first DMA is in flight. Only the
            # first term's exact region (p 0:112, mu 0:7, mv 0:7, qs 1:8)
            # gates the first add -- it goes first on DVE. The two pieces
            # with large free size go to the Pool engine (memset is its
            # one efficient op) between its first DMA issues; DVE memset
            # cost scales with free size, not partitions.
            au = acc[:, :].bitcast(mybir.dt.uint32).rearrange(
                "p (mu mv qs w) -> p mu mv qs w", mu=8, mv=8, qs=8, w=6,
            )
            nc.vector.memset(au[0:112, 0:7, 0:7, 1:8, :], 0)
            for ti, (e, f, a, b) in enumerate(TERMS):
                isN, itN = 8 - e, 8 - f
                fd = NU * NV * isN * 12
                if (e, f, a, b) in INT8_TERMS:
                    st = stg8p.tile([P128, NU * NV * 8 * 12], mybir.dt.int8,
                                    tag="stg8")
                else:
                    st = stgp.tile([P128, NU * NV * 8 * 12], mybir.dt.float16,
                                   tag="stg")
                # SWDGE (Pool-issued) DMA casts int8 -> fp16 in flight
                # (no cast for the int8-staged terms). The very first
                # term's DMA is split by iu so its first mu-subops start
                # ~2us before the full term lands.
                if ti == 0:
                    # First chunk rides the otherwise-idle HWDGE (sync)
                    # queue so the first add starts before the SWDGE
                    # pipeline spins up.
                    colsplit = NV * isN * 12 * 2
                    nc.sync.dma_start(out=st[:itN * 16, :colsplit],
                                      in_=tfirst.ap())
                    nc.gpsimd.dma_start(out=st[:itN * 16, colsplit:fd],
                                        in_=terms[(e, f, a, b)].ap()[:, colsplit:])
                else:
                    nc.gpsimd.dma_start(out=st[:itN * 16, :fd],
                                        in_=terms[(e, f, a, b)].ap())
                if ti == 0:
                    # Zero partitions 112:128 by DMA on the idle sync
                    # queue (first f=0 add is term 5); the qs=0 strip on
                    # Pool (first e=0 add is term 9).
                    nc.sync.dma_start(out=acc[112:128, :], in_=zrows.ap())
                elif ti == 1:
                    nc.gpsimd.memset(au[0:112, 0:7, 0:7, 0:1, :], 0)
                sv = st[:itN * 16, :fd].rearrange(
                    "p (iu iv qs ru rv c) -> p iu iv qs ru rv c",
                    iu=NU, iv=NV, qs=isN, ru=2, rv=2, c=3,
                )
                last = ti == len(TERMS) - 1
                if e == 0 and ti == len(TERMS) - 2:
                    # second-to-last term A=(0,1,1,1): emit only its low
                    # mu piece now; the high piece interleaves after the
                    # last term's low piece so out1 issues one add sooner.
                    ovA1 = accv[0:itN * 16, 1:5, b:b + 7, :, :, :, :]
                    nc.vector.tensor_add(out=ovA1, in0=ovA1, in1=sv[:, 0:4])
                    svA, itNA, bA = sv, itN, b
                elif e == 0 and not last:
                    # free AP collapses to <= 3 dims: one op per term
                    ov = accv[0:itN * 16, a:a + 7, b:b + 7, e:8, :, :, :]
                    nc.vector.tensor_add(out=ov, in0=ov, in1=sv)
                elif e == 0 and last:
                    # final term B=(0,0,1,1): B-low, out1, then the two
                    # deferred high pieces, out2; output halves ride
                    # different queues so they overlap the final adds.
                    ovB1 = accv[:, 1:5, b:b + 7, :, :, :, :]
                    nc.vector.tensor_add(out=ovB1, in0=ovB1, in1=sv[:, 0:4])
                    nc.sync.dma_start(out=out.ap()[:, :5 * 768],
                                      in_=acc[:, :5 * 768])
                    ovA2 = accv[0:itNA * 16, 5:8, bA:bA + 7, :, :, :, :]
                    nc.vector.tensor_add(out=ovA2, in0=ovA2, in1=svA[:, 4:7])
                    ovB2 = accv[:, 5:8, b:b + 7, :, :, :, :]
                    nc.vector.tensor_add(out=ovB2, in0=ovB2, in1=sv[:, 4:7])
                    nc.gpsimd.dma_start(out=out.ap()[:, 5 * 768:],
                                        in_=acc[:, 5 * 768:])
                else:
                    # qs-clipped terms need 4 free dims; the ISA caps free
                    # APs at 3 dims, so loop mu.
                    for iu in range(NU):
                        ovi = accv[0:itN * 16,
                                   a + iu:a + iu + 1, b:b + 7, 1:8, :, :, :]
                        svi = sv[:, iu:iu + 1]
                        nc.vector.tensor_add(out=ovi, in0=ovi, in1=svi)
                if ti == 0:
                    # mv=7 strip: needed by term 2 (b=1)
                    nc.vector.memset(au[0:112, 0:7, 7:8, :, :], 0)
                elif ti == 1:
                    # mu=7 strip: needed by term 3 (a=1)
                    nc.vector.memset(au[0:112, 7:8, :, :, :], 0)
    nc.compile()
    return nc


def kernel(x):
    x = np.ascontiguousarray(np.asarray(x), dtype=np.float32)
    in_maps, scales = _shard(x)
    nc = build_nc()
    from concourse.bass_utils import run_bass_kernel_spmd

    res = run_bass_kernel_spmd(nc, in_maps, core_ids=list(range(8)))
    return _assemble([r["out"] for r in res.results], scales)


# revision 40
# speedup vs baseline: 1.0439x; 1.0439x over previous
"""DePatchEfficient Trainium2 kernel.

Reconstructs a (B, U, V, S, T, C) volume from overlapping 4D patches by
scatter-add + overlap-count division (overlap-add).

Decomposition: polyphase overlap-add. Split patch offsets ju = 2a + ru,
jv = 2b + rv, js = 4e + ws, jt = 4f + wt. Then every output element
  out[u=2mu+ru, v=2mv+rv, s=4qs+ws, t=4qt+wt, c]
is the sum over 16 terms (a, b, e, f) of shifted input slabs with
mu = iu + a, mv = iv + b, qs = is + e, qt = it + f.

Design notes (measured on HW: DMA engine time is paced by SBUF-landed
bytes at ~19.5 GB/s/engine x 16 engines; DVE adds at ~2 elem/cyc fp16):
- Inputs ship as int8: q = round(x / s) with one scale s per core. The
  Pool-engine (SWDGE) DMA casts int8 -> fp16 in flight. All on-core
  adds are integer-valued fp16 sums bounded by 16*127 = 2032 < 2048 --
  EXACT in fp16. The only error is input quantization (~s/2 = 0.02 abs,
  ~5.7e-3 rel vs the output max), well under the 2e-2 gate.
- Both the scale s and the 1/overlap-count division fold into the
  host-side assembly (a single fused multiply on the fp32 result).
- Two e=0 terms stage as raw int8 in SBUF: their DMA moves half the
  bytes while their adds drop to DVE 1x mode -- balances the DMA stream
  (the pacer) against DVE slack.
- Partition dim is (qt', ws, wt) = 128 with qt' = 7 - qt: the f-shift
  becomes a partition-range restriction starting at 0 (SBUF APs cannot
  start at partition 16), so only the e-shift still forces the free AP
  over 3 dims (e=1 terms loop over mu).
- The acc is zeroed by one DVE memset in the dead window while the
  first DMA is in flight; every term is then a plain add. The first
  term's DMA is split by iu so adds start early; the final term's add
  is split by mu so the output DMA pipelines behind the last adds.

Sharding: 8 cores = (batch b2) x (s-half) x (t-half); high halves are
axis-flipped host-side so all cores run an identical program. Halo
elements are clipped host-side: each input element ships to one core.

On-core layout (SBUF): partitions = (qt'8, ws4, wt4) = 128; free dim =
(mu8, mv8, qs8, ru2, rv2, c3) = 6144 fp16. ru/rv/c form a never-sliced
contiguous 12-elem inner block, keeping fp16 adds in DVE 2x_1p mode.
"""

import os
import sys

import numpy as np

for _p in ("/opt/trn_rl_repo",):
    if os.path.isdir(_p) and _p not in sys.path:
        sys.path.insert(0, _p)

B, U, V, S, T, C = 2, 16, 16, 64, 64, 3
NS, NT, NU, NV = 15, 15, 7, 7
P128 = 128        # partitions: (qt'8, ws4, wt4)
FREE = 6144       # free dim: (mu8, mv8, qs8, ru2, rv2, c3)

# The acc is zeroed by one full memset in the dead DVE window while the
# first DMA is in flight, so every term is a plain add in any order.
# Smallest terms (e=1, f=1) go first so the first add starts earliest;
# (0,0,1,1) is last: its add is mu-split so the output DMA pipelines
# behind the final adds.
TERMS = [
    (1, 1, 0, 0), (1, 1, 0, 1), (1, 1, 1, 0), (1, 1, 1, 1),
    (1, 0, 0, 0), (1, 0, 0, 1), (1, 0, 1, 0), (1, 0, 1, 1),
    (0, 0, 0, 0), (0, 0, 0, 1), (0, 0, 1, 0), (0, 1, 0, 0),
    (0, 1, 0, 1), (0, 1, 1, 0), (0, 1, 1, 1), (0, 0, 1, 1),
]

# Terms staged as raw int8 in SBUF: their DMA moves half the bytes
# (the stream is dest-byte-paced) while their DVE add drops to 1x mode
# — DVE has slack, the DMA stream is the critical path. Balanced at ~2
# single-op (e=0) terms.
INT8_TERMS = {(0, 0, 0, 1), (0, 1, 0, 0)}

# Per-axis overlap counts in core-local coordinates (after flips every
# core sees the volume edge at index 0; u/v are unsharded so both ends
# are edges).
_CU = np.array([1, 2, 2, 2, 2, 2, 2, 1], np.float32)   # mu / mv
_CS = np.array([1, 2, 2, 2, 2, 2, 2, 2], np.float32)   # qs / qt


def _term_name(e, f, a, b):
    return f"t{e}{f}{a}{b}"


def _inv_count_map():
    """1/count over the core-local assembled slab (U, V, 32, 32, 1)."""
    cu = _CU[np.arange(U) // 2]            # (16,) by u
    cs = _CS[np.arange(32) // 4]           # (32,) by s_local
    cnt = (cu[:, None, None, None] * cu[None, :, None, None]
           * cs[None, None, :, None] * cs[None, None, None, :])
    return (1.0 / cnt[..., None]).astype(np.float32)


_INV_COUNT = _inv_count_map()


def _shard(x):
    """Full input (B, 11025, 4, 4, 8, 8, 3) -> (per-core in_maps, scales).

    Per core: quantize to int8 with scale s = max|x_core|/127; terms are
    staged so the SWDGE cast-DMA lands them as fp16 integer values."""
    x9 = np.ascontiguousarray(x).reshape(B, NS, NT, NU, NV, 4, 4, 8, 8, C)
    in_maps, scales = [], []
    for core in range(8):
        b, sh, th = core // 4, (core // 2) % 2, core % 2
        xc = x9[b, 7 * sh:7 * sh + 8, 7 * th:7 * th + 8]
        # Flip high-half cores so every core sees an "s/t low half" problem.
        if sh:
            xc = xc[::-1, :, :, :, :, :, ::-1]
        if th:
            xc = xc[:, ::-1, :, :, :, :, :, ::-1]
        s = float(np.abs(xc).max()) / 127.0
        # (is, it, iu, iv, a ru, b rv, e ws, f wt, c)
        xq = np.clip(np.rint(xc * (1.0 / s)), -127, 127).astype(np.int8)
        xq = xq.reshape(8, 8, NU, NV, 2, 2, 2, 2, 2, 4, 2, 4, C)
        scales.append(s)
        m = {}
        for (e, f, a, bb) in TERMS:
            isN, itN = 8 - e, 8 - f
            sl = xq[:isN, :itN, :, :, a, :, bb, :, e, :, f, :, :]
            # it axis reversed: acc partitions hold qt' = 7 - qt, so every
            # term's partition window starts at 0.
            sl = sl[:, ::-1]
            # (is,it',iu,iv,ru,rv,ws,wt,c) -> (it',ws,wt,iu,iv,is,ru,rv,c)
            sl = sl.transpose(1, 6, 7, 2, 3, 0, 4, 5, 8)
            m[_term_name(e, f, a, bb)] = np.ascontiguousarray(
                sl.reshape(itN * 16, NU * NV * isN * 12)
            )
        # First term's first 3 iu-blocks also ship as fp16 for the HWDGE
        # (sync) queue, which cannot cast: the first add starts before
        # the SWDGE pipeline spins up.
        e0, f0, a0, b0 = TERMS[0]
        cs0 = NV * (8 - e0) * 12 * 2
        m["t_first"] = m[_term_name(e0, f0, a0, b0)][:, :cs0].astype(np.float16)
        in_maps.append(m)
    return in_maps, scales


def _assemble(core_outs, scales):
    """Per-core (128, 6144) fp16 int-sums -> full (B, U, V, S, T, C) fp32."""
    full = np.empty((B, U, V, S, T, C), np.float32)
    for core in range(8):
        b, sh, th = core // 4, (core // 2) % 2, core % 2
        o = core_outs[core].reshape(8, 4, 4, 8, 8, 8, 2, 2, C)[::-1]
        # (qt, ws, wt, mu, mv, qs, ru, rv, c) -> (mu ru, mv rv, qs ws, qt wt, c)
        o = o.transpose(3, 6, 4, 7, 5, 1, 0, 2, 8).reshape(U, V, 32, 32, C)
        o = o.astype(np.float32) * (scales[core] * _INV_COUNT)
        if sh:
            o = o[:, :, ::-1]
        if th:
            o = o[:, :, :, ::-1]
        full[b, :, :, 32 * sh:32 * sh + 32, 32 * th:32 * th + 32, :] = o
    return full


def build_nc(reps=1):
    """Build the per-core Bass program (identical for all 8 cores).

    reps>1 wraps the whole pass in a Tile For_i loop executing it `reps`
    times back-to-back — benchmark-only; the graded kernel() path uses
    reps=1 with no loop.
    """
    import concourse.bacc as bacc
    import concourse.mybir as mybir
    from concourse.tile import TileContext

    nc = bacc.Bacc("TRN2", target_bir_lowering=False, debug=False)
    terms = {
        (e, f, a, b): nc.dram_tensor(
            _term_name(e, f, a, b),
            [(8 - f) * 16, NU * NV * (8 - e) * 12],
            mybir.dt.int8,
            kind="ExternalInput",
        )
        for (e, f, a, b) in TERMS
    }
    e0, f0 = TERMS[0][0], TERMS[0][1]
    tfirst = nc.dram_tensor(
        "t_first", [(8 - f0) * 16, NV * (8 - e0) * 12 * 2], mybir.dt.float16,
        kind="ExternalInput",
    )
    out = nc.dram_tensor("out", [P128, FREE], mybir.dt.float16, kind="ExternalOutput")

    from contextlib import ExitStack

    with (
        TileContext(nc) as tc,
        tc.tile_pool(name="accp", bufs=1) as accp,
        tc.tile_pool(name="stgp", bufs=6) as stgp,
        tc.tile_pool(name="stg8p", bufs=2) as stg8p,
        ExitStack() as stack,
    ):
        if reps > 1:
            stack.enter_context(tc.For_i(0, reps, 1))
        if True:
            acc = accp.tile([P128, FREE], mybir.dt.float16)
            accv = acc[:, :].rearrange(
                "p (mu mv qs ru rv c) -> p mu mv qs ru rv c",
                mu=8, mv=8, qs=8, ru=2, rv=2, c=3,
            )
            # Zero the whole acc while the first DMA is in flight (DVE is
            # otherwise idle until the first term lands).
            nc.vector.memset(acc[:, :].bitcast(mybir.dt.uint32), 0)
            for ti, (e, f, a, b) in enumerate(TERMS):
                isN, itN = 8 - e, 8 - f
                fd = NU * NV * isN * 12
                if (e, f, a, b) in INT8_TERMS:
                    st = stg8p.tile([P128, NU * NV * 8 * 12], mybir.dt.int8,
                                    tag="stg8")
                else:
                    st = stgp.tile([P128, NU * NV * 8 * 12], mybir.dt.float16,
                                   tag="stg")
                # SWDGE (Pool-issued) DMA casts int8 -> fp16 in flight
                # (no cast for the int8-staged terms). The very first
                # term's DMA is split by iu so its first mu-subops start
                # ~2us before the full term lands.
                if ti == 0:
                    # First chunk rides the otherwise-idle HWDGE (sync)
                    # queue so the first add starts before the SWDGE
                    # pipeline spins up.
                    colsplit = NV * isN * 12 * 2
                    nc.sync.dma_start(out=st[:itN * 16, :colsplit],
                                      in_=tfirst.ap())
                    nc.gpsimd.dma_start(out=st[:itN * 16, colsplit:fd],
                                        in_=terms[(e, f, a, b)].ap()[:, colsplit:])
                else:
                    nc.gpsimd.dma_start(out=st[:itN * 16, :fd],
                                        in_=terms[(e, f, a, b)].ap())
                sv = st[:itN * 16, :fd].rearrange(
                    "p (iu iv qs ru rv c) -> p iu iv qs ru rv c",
                    iu=NU, iv=NV, qs=isN, ru=2, rv=2, c=3,
                )
                last = ti == len(TERMS) - 1
                if e == 0 and ti == len(TERMS) - 2:
                    # second-to-last term (0,1,1,1): mu-split so the low
                    # half of the output is final one add earlier.
                    ov1 = accv[0:itN * 16, 1:5, b:b + 7, :, :, :, :]
                    nc.vector.tensor_add(out=ov1, in0=ov1, in1=sv[:, 0:4])
                    ov2 = accv[0:itN * 16, 5:8, b:b + 7, :, :, :, :]
                    nc.vector.tensor_add(out=ov2, in0=ov2, in1=sv[:, 4:7])
                elif e == 0 and not last:
                    # free AP collapses to <= 3 dims: one op per term
                    ov = accv[0:itN * 16, a:a + 7, b:b + 7, e:8, :, :, :]
                    nc.vector.tensor_add(out=ov, in0=ov, in1=sv)
                elif e == 0 and last:
                    # final term (0,0,1,1): split the add by mu so the
                    # output DMA pipelines behind the last adds.
                    ov1 = accv[:, 1:5, b:b + 7, :, :, :, :]
                    nc.vector.tensor_add(out=ov1, in0=ov1, in1=sv[:, 0:4])
                    nc.sync.dma_start(out=out.ap()[:, :5 * 768],
                                      in_=acc[:, :5 * 768])
                    ov2 = accv[:, 5:8, b:b + 7, :, :, :, :]
                    nc.vector.tensor_add(out=ov2, in0=ov2, in1=sv[:, 4:7])
                    # second output half on the (now idle) SWDGE queue so
                    # the two output DMAs overlap.
                    nc.gpsimd.dma_start(out=out.ap()[:, 5 * 768:],
                                        in_=acc[:, 5 * 768:])
                else:
                    # qs-clipped terms need 4 free dims; the ISA caps free
                    # APs at 3 dims, so loop mu.
                    for iu in range(NU):
                        ovi = accv[0:itN * 16,
                                   a + iu:a + iu + 1, b:b + 7, 1:8, :, :, :]
                        svi = sv[:, iu:iu + 1]
                        nc.vector.tensor_add(out=ovi, in0=ovi, in1=svi)
    nc.compile()
    return nc


def kernel(x):
    x = np.ascontiguousarray(np.asarray(x), dtype=np.float32)
    in_maps, scales = _shard(x)
    nc = build_nc()
    from concourse.bass_utils import run_bass_kernel_spmd

    res = run_bass_kernel_spmd(nc, in_maps, core_ids=list(range(8)))
    return _assemble([r["out"] for r in res.results], scales)
